# revision 1
# baseline (speedup 1.0000x reference)
"""Trainium2 Bass kernel for nn_HGNER (windowed bi-LSTM + attention + linear head).

Sharding: 8 cores x 128 tokens (data-parallel over the flattened (B,L) token
axis; each core gets half of one batch row plus a 4-token halo). Small LSTM /
linear params are replicated to every core as bf16.

Layout inside a core: "feature-partition" — SBUF partitions carry a 128-wide
feature chunk, the free dim packs (chunk, token). This makes the recurrent
h @ W_hh matmuls transpose-free (h free-slices are directly the moving
operand) and keeps elementwise ops on long free dims.

Per (window, direction) chain, the input projection G = x @ W_ih.T + b is
computed ONCE over the 136-token halo range; each LSTM step injects a
token-shifted slice of G into PSUM via an identity matmul and accumulates the
h @ W_hh part on top, so the x-projection cost is amortized across the w
steps of the window.
"""

import numpy as np
import ml_dtypes

import concourse.bass as bass
import concourse.bacc as bacc_mod
import concourse.mybir as mybir
from concourse.tile import TileContext
from concourse.bass_utils import run_bass_kernel_spmd

F32 = mybir.dt.float32
BF16 = mybir.dt.bfloat16
AF = mybir.ActivationFunctionType
ALU = mybir.AluOpType
AX = mybir.AxisListType

B, L, D, H, NW, NL = 4, 256, 768, 384, 4, 9
WINDOWS = (3, 5, 7, 9)
NCORES = 8
TPC = 128          # tokens per core
HALO = 4           # max half-window
TH = TPC + 2 * HALO  # 136 tokens incl. halo
DC = D // 128      # 6 chunks of input features
HC = H // 128      # 3 chunks of hidden features
GC = 4 * H // 128  # 12 chunks of gate features
NCH = 2 * NW       # 8 (window, direction) chains
SCALE = 1.0 / np.sqrt(np.float32(D))

_CACHE = {}


def _build():
    nc = bacc_mod.Bacc()

    # ---- DRAM I/O ----
    xt_d = nc.dram_tensor("xt", [D, TH], BF16, kind="ExternalInput")
    xc32_d = nc.dram_tensor("xc32", [D, TPC], F32, kind="ExternalInput")
    masks_d = nc.dram_tensor("masks", [9, 128, HC * TPC], mybir.dt.uint8, kind="ExternalInput")
    wih_d = nc.dram_tensor("wih", [NCH, D, 4 * H], BF16, kind="ExternalInput")
    whh_d = nc.dram_tensor("whh", [NCH, H, 4 * H], BF16, kind="ExternalInput")
    bias_d = nc.dram_tensor("bias", [NCH, 1, 4 * H], BF16, kind="ExternalInput")
    linw_d = nc.dram_tensor("linw", [D, NL], BF16, kind="ExternalInput")
    linb_d = nc.dram_tensor("linb", [1, NL], BF16, kind="ExternalInput")
    identb_d = nc.dram_tensor("identb", [128, 128], BF16, kind="ExternalInput")
    ident32_d = nc.dram_tensor("ident32", [128, 128], F32, kind="ExternalInput")
    ones_d = nc.dram_tensor("ones", [128, 1], BF16, kind="ExternalInput")
    onesr_d = nc.dram_tensor("onesr", [1, 512], BF16, kind="ExternalInput")
    out_d = nc.dram_tensor("out", [NL, TPC], F32, kind="ExternalOutput")

    with TileContext(nc) as tc:
        with (
            tc.tile_pool(name="const", bufs=1) as cpool,
            tc.tile_pool(name="wih", bufs=2) as wih_pool,
            tc.tile_pool(name="whh", bufs=6) as whh_pool,
            tc.tile_pool(name="g", bufs=8) as g_pool,
            tc.tile_pool(name="muti", bufs=NCH + 2) as muti_pool,
            tc.tile_pool(name="st", bufs=2) as st_pool,
            tc.tile_pool(name="tmp", bufs=3) as tmp_pool,
            tc.tile_pool(name="fin", bufs=2) as fin_pool,
            tc.tile_pool(name="ps", bufs=1, space="PSUM") as ps_pool,
            tc.tile_pool(name="psg", bufs=2, space="PSUM") as psg_pool,
        ):
            # ---- load constants ----
            xt = cpool.tile([128, DC * TH], BF16, tag="xt")
            nc.sync.dma_start(
                out=xt[:].rearrange("p (k t) -> p k t", t=TH),
                in_=xt_d[:].rearrange("(k p) t -> p k t", p=128),
            )
            xc32 = cpool.tile([128, DC * TPC], F32, tag="xc32")
            nc.sync.dma_start(
                out=xc32[:].rearrange("p (k t) -> p k t", t=TPC),
                in_=xc32_d[:].rearrange("(k p) t -> p k t", p=128),
            )
            masks = cpool.tile([128, 9 * HC * TPC], mybir.dt.uint8, tag="masks")
            nc.sync.dma_start(
                out=masks[:].rearrange("p (o t) -> p o t", o=9),
                in_=masks_d[:].rearrange("o p t -> p o t"),
            )
            biasr = cpool.tile([1, NCH * 4 * H], BF16, tag="bias")
            nc.sync.dma_start(
                out=biasr[:].rearrange("o (c n) -> o c n", c=NCH),
                in_=bias_d[:].rearrange("c o n -> o c n"),
            )
            linw = cpool.tile([128, DC * NL], BF16, tag="linw")
            nc.sync.dma_start(
                out=linw[:].rearrange("p (k n) -> p k n", n=NL),
                in_=linw_d[:].rearrange("(k p) n -> p k n", p=128),
            )
            linb = cpool.tile([1, NL], BF16, tag="linb")
            nc.sync.dma_start(out=linb[:], in_=linb_d[:])
            identb = cpool.tile([128, 128], BF16, tag="identb")
            nc.sync.dma_start(out=identb[:], in_=identb_d[:])
            ident32 = cpool.tile([128, 128], F32, tag="ident32")
            nc.sync.dma_start(out=ident32[:], in_=ident32_d[:])
            ones = cpool.tile([128, 1], BF16, tag="ones")
            nc.sync.dma_start(out=ones[:], in_=ones_d[:])
            onesr = cpool.tile([1, 512], BF16, tag="onesr")
            nc.sync.dma_start(out=onesr[:], in_=onesr_d[:])

            # one-time DVE touches of DMA-loaded consts: collapse later DVE
            # waits to a single semaphore (DVE instr structs have 1 wait slot)
            wu8 = cpool.tile([128, 1], mybir.dt.uint8, tag="wu8")
            nc.vector.tensor_copy(wu8[:], masks[:, 0:1])
            wf0 = cpool.tile([128, 1], F32, tag="wf0")
            nc.vector.tensor_copy(wf0[:], xc32[:, 0:1])
            wb0 = cpool.tile([128, 1], BF16, tag="wb0")
            nc.vector.tensor_copy(wb0[:], xt[:, 0:1])

            mutis = []  # final h per chain, [128, HC*TPC] bf16

            for wi, w in enumerate(WINDOWS):
                half = w // 2
                gs = []
                whhs = []
                for d in (0, 1):
                    c = wi * 2 + d
                    # stream weights for this chain
                    wih = wih_pool.tile([128, DC * 4 * H], BF16, tag="wih")
                    nc.sync.dma_start(
                        out=wih[:].rearrange("p (k n) -> p k n", k=DC),
                        in_=wih_d[c].rearrange("(k p) n -> p k n", p=128),
                    )
                    whh = whh_pool.tile([128, HC * 4 * H], BF16, tag="whh")
                    nc.sync.dma_start(
                        out=whh[:].rearrange("p (k n) -> p k n", k=HC),
                        in_=whh_d[c].rearrange("(k p) n -> p k n", p=128),
                    )
                    whhs.append(whh)
                    # ---- G precompute: [128, GC*TH] bf16, bias folded in ----
                    g = g_pool.tile([128, GC * TH], BF16, tag="g")
                    for j in range(GC):
                        ps = psg_pool.tile([128, TH], F32, tag="gps")
                        for k in range(DC):
                            nc.tensor.matmul(
                                ps[:],
                                lhsT=wih[:, k * 4 * H + j * 128:k * 4 * H + (j + 1) * 128],
                                rhs=xt[:, k * TH:(k + 1) * TH],
                                start=(k == 0),
                                stop=False,
                            )
                        nc.tensor.matmul(
                            ps[:],
                            lhsT=biasr[:, c * 4 * H + j * 128:c * 4 * H + (j + 1) * 128],
                            rhs=onesr[:, 0:TH],
                            start=False,
                            stop=True,
                        )
                        nc.vector.tensor_copy(g[:, j * TH:(j + 1) * TH], ps[:])
                    gs.append(g)

                # ---- run both directions' chains, step-interleaved ----
                cst = [st_pool.tile([128, HC * TPC], BF16, tag=f"c{d}", name=f"cst{wi}_{d}") for d in (0, 1)]
                hst = [muti_pool.tile([128, HC * TPC], BF16, tag="muti", name=f"hst{wi}_{d}") for d in (0, 1)]
                for d in (0, 1):
                    nc.vector.memset(cst[d][:], 0.0)
                    nc.vector.memset(hst[d][:], 0.0)

                for t in range(w):
                    for d in (0, 1):
                        o = (t - half) if d == 0 else (half - t)
                        g = gs[d]
                        whh = whhs[d]
                        gps = ps_pool.tile([128, 4 * H], F32, tag=f"gates{d}")
                        g3 = g[:].rearrange("p (j t) -> p j t", t=TH)
                        # inject shifted G slice (3 banks of 512)
                        for nb in range(3):
                            nc.tensor.matmul(
                                gps[:, nb * 512:(nb + 1) * 512],
                                lhsT=identb[:],
                                rhs=g3[:, nb * 4:(nb + 1) * 4, HALO + o:HALO + o + TPC],
                                start=True,
                                stop=(t == 0),
                            )
                        if t > 0:
                            # gates += W_hh @ h   (feature-partition both sides)
                            for j in range(GC):
                                for k in range(HC):
                                    nc.tensor.matmul(
                                        gps[:, j * 128:(j + 1) * 128],
                                        lhsT=whh[:, k * 4 * H + j * 128:k * 4 * H + (j + 1) * 128],
                                        rhs=hst[d][:, k * TPC:(k + 1) * TPC],
                                        start=False,
                                        stop=(k == HC - 1),
                                    )
                        # activations (gate order i,f,g,o)
                        sif = tmp_pool.tile([128, 2 * H], BF16, tag="sif")
                        nc.scalar.activation(sif[:], gps[:, 0:2 * H], AF.Sigmoid)
                        tg = tmp_pool.tile([128, H], BF16, tag="tg")
                        nc.scalar.activation(tg[:], gps[:, 2 * H:3 * H], AF.Tanh)
                        so = tmp_pool.tile([128, H], BF16, tag="so")
                        nc.scalar.activation(so[:], gps[:, 3 * H:4 * H], AF.Sigmoid)
                        # c_new = sig(f)*c + sig(i)*tanh(g)
                        cn = tmp_pool.tile([128, H], BF16, tag="cn")
                        if t > 0:
                            fc = tmp_pool.tile([128, H], BF16, tag="fc")
                            nc.vector.tensor_tensor(fc[:], sif[:, H:2 * H], cst[d][:], ALU.mult)
                            ig = tmp_pool.tile([128, H], BF16, tag="ig")
                            nc.vector.tensor_tensor(ig[:], sif[:, 0:H], tg[:], ALU.mult)
                            nc.vector.tensor_tensor(cn[:], ig[:], fc[:], ALU.add)
                        else:
                            nc.vector.tensor_tensor(cn[:], sif[:, 0:H], tg[:], ALU.mult)
                        tcn = tmp_pool.tile([128, H], BF16, tag="tcn")
                        nc.scalar.activation(tcn[:], cn[:], AF.Tanh)
                        hn = tmp_pool.tile([128, H], BF16, tag="hn")
                        nc.vector.tensor_tensor(hn[:], so[:], tcn[:], ALU.mult)
                        # masked state update (invalid steps keep old state)
                        mk = masks[:, (o + HALO) * H:(o + HALO + 1) * H]
                        nc.vector.copy_predicated(cst[d][:], mk, cn[:])
                        nc.vector.copy_predicated(hst[d][:], mk, hn[:])
                mutis.extend(hst)

            # ---- attention over the 4 window features ----
            xt3 = xt[:].rearrange("p (k t) -> p k t", t=TH)
            score_ps = psg_pool.tile([128, NW], F32, tag="gps")
            prods = []
            for wi in range(NW):
                for d in (0, 1):
                    pr = tmp_pool.tile([128, HC * TPC], BF16, tag=f"pr{d}")
                    nc.vector.tensor_tensor(
                        pr[:],
                        mutis[wi * 2 + d][:],
                        xt3[:, d * HC:(d + 1) * HC, HALO:HALO + TPC],
                        ALU.mult,
                    )
                    prods.append(pr)
                for ci in range(2 * HC):
                    pr = prods[wi * 2 + ci // HC]
                    k = ci % HC
                    nc.tensor.matmul(
                        score_ps[:, wi:wi + 1],
                        lhsT=pr[:, k * TPC:(k + 1) * TPC],
                        rhs=ones[:],
                        start=(ci == 0),
                        stop=(ci == 2 * HC - 1),
                    )
            # softmax over the NW axis (token-partition [128, 4])
            mx = tmp_pool.tile([128, 1], F32, tag="mx")
            nc.vector.reduce_max(mx[:], score_ps[:], axis=AX.X)
            mxn = tmp_pool.tile([128, 1], F32, tag="mxn")
            nc.vector.tensor_scalar(mxn[:], mx[:], float(-SCALE), None, ALU.mult)
            ex = tmp_pool.tile([128, NW], F32, tag="ex")
            nc.scalar.activation(ex[:], score_ps[:], AF.Exp, bias=mxn[:], scale=float(SCALE))
            sm = tmp_pool.tile([128, 1], F32, tag="sm")
            nc.vector.reduce_sum(sm[:], ex[:], axis=AX.X)
            rs = tmp_pool.tile([128, 1], F32, tag="rs")
            nc.vector.reciprocal(rs[:], sm[:])
            attn = tmp_pool.tile([128, NW], BF16, tag="attn")
            nc.vector.tensor_scalar(attn[:], ex[:], rs[:], None, ALU.mult)
            # per-window: transpose attn column to [1,128], replicate to [1,384],
            # outer-product with a ones column to broadcast over partitions
            bcs = []
            for wi in range(NW):
                at_ps = psg_pool.tile([1, TPC], BF16, tag="gps", name=f"atps{wi}")
                nc.tensor.transpose(at_ps[:], attn[:, wi:wi + 1], identb[:])
                at_sb = tmp_pool.tile([1, HC * TPC], BF16, tag="atsb", name=f"atsb{wi}")
                for k in range(HC):
                    nc.vector.tensor_copy(at_sb[:, k * TPC:(k + 1) * TPC], at_ps[:])
                bc_ps = psg_pool.tile([128, HC * TPC], F32, tag="gps", name=f"bcps{wi}")
                nc.tensor.matmul(
                    bc_ps[:], lhsT=onesr[:, 0:128], rhs=at_sb[:], start=True, stop=True,
                )
                bc = tmp_pool.tile([128, HC * TPC], BF16, tag="bc", name=f"bc{wi}", bufs=NW)
                nc.vector.tensor_copy(bc[:], bc_ps[:])
                bcs.append(bc)
            accs = []
            for d in (0, 1):
                acc = fin_pool.tile([128, HC * TPC], F32, tag=f"acc{d}")
                for wi in range(NW):
                    bc = bcs[wi]
                    t1 = tmp_pool.tile([128, HC * TPC], F32, tag="t1")
                    nc.vector.tensor_tensor(t1[:], mutis[wi * 2 + d][:], bc[:], ALU.mult)
                    if wi == 0:
                        nc.vector.tensor_copy(acc[:], t1[:])
                    else:
                        nc.vector.tensor_tensor(acc[:], acc[:], t1[:], ALU.add)
                # residual: out = x + local_feat  (fp32 x for precision)
                nc.vector.tensor_tensor(
                    acc[:], acc[:],
                    xc32[:, d * HC * TPC:(d + 1) * HC * TPC], ALU.add,
                )
                accb = fin_pool.tile([128, HC * TPC], BF16, tag=f"accb{d}")
                nc.vector.tensor_copy(accb[:], acc[:])
                accs.append(accb)
            # ---- linear head: logits [9, 128] ----
            lg_ps = psg_pool.tile([NL, TPC], F32, tag="gps")
            for ci in range(DC):
                d = ci // HC
                k = ci % HC
                nc.tensor.matmul(
                    lg_ps[:],
                    lhsT=linw[:, ci * NL:(ci + 1) * NL],
                    rhs=accs[d][:, k * TPC:(k + 1) * TPC],
                    start=(ci == 0),
                    stop=False,
                )
            nc.tensor.matmul(
                lg_ps[:], lhsT=linb[:], rhs=onesr[:, 0:TPC],
                start=False, stop=True,
            )
            ob = fin_pool.tile([NL, TPC], F32, tag="ob")
            nc.vector.tensor_copy(ob[:], lg_ps[:])
            nc.sync.dma_start(out=out_d[:], in_=ob[:])

    nc.finalize()
    return nc


def _valid_scatter_np(x, valid_ids):
    Bx, Lx, Dx = x.shape
    v = (valid_ids == 1)
    out = np.zeros_like(x)
    for b in range(Bx):
        sel = x[b][v[b]]
        out[b, :sel.shape[0]] = sel
    return out


def _host_prep(inputs):
    seq_out = np.asarray(inputs["seq_out"], np.float32)
    valid_ids = np.asarray(inputs["valid_ids"])
    x = _valid_scatter_np(seq_out, valid_ids)  # [B,L,D] f32

    bf = ml_dtypes.bfloat16
    # weights, chain order c = window_idx*2 + dir (0=f, 1=b)
    wih = np.empty((NCH, D, 4 * H), bf)
    whh = np.empty((NCH, H, 4 * H), bf)
    biasv = np.empty((NCH, 1, 4 * H), bf)
    for wi in range(NW):
        for d, sfx in ((0, "f"), (1, "b")):
            c = wi * 2 + d
            wih[c] = np.asarray(inputs[f"w_ih_{sfx}"][wi], np.float32).T.astype(bf)
            whh[c] = np.asarray(inputs[f"w_hh_{sfx}"][wi], np.float32).T.astype(bf)
            bv = (np.asarray(inputs[f"b_ih_{sfx}"][wi], np.float32)
                  + np.asarray(inputs[f"b_hh_{sfx}"][wi], np.float32))
            biasv[c] = bv[None, :].astype(bf)
    linw = np.asarray(inputs["lin_w"], np.float32).T.astype(bf)  # [768, 9]
    linb = np.asarray(inputs["lin_b"], np.float32)[None, :].astype(bf)
    identb = np.eye(128, dtype=bf)
    ident32 = np.eye(128, dtype=np.float32)
    ones = np.ones((128, 1), bf)

    in_maps = []
    for core in range(NCORES):
        b = core // 2
        t0 = (core % 2) * TPC
        # halo slice [t0-4, t0+132) of row b, zero-padded outside [0, L)
        xh = np.zeros((TH, D), np.float32)
        lo = max(0, t0 - HALO)
        hi = min(L, t0 + TPC + HALO)
        xh[lo - (t0 - HALO):hi - (t0 - HALO)] = x[b, lo:hi]
        xt = np.ascontiguousarray(xh.T).astype(bf)            # [768, 136]
        xc32 = np.ascontiguousarray(x[b, t0:t0 + TPC].T)      # [768, 128] f32
        mk = np.empty((9, 128, HC * TPC), np.uint8)
        for o in range(-HALO, HALO + 1):
            tg = t0 + np.arange(TPC) + o
            m = ((tg >= 0) & (tg < L)).astype(np.uint8)
            mk[o + HALO] = np.broadcast_to(np.tile(m, HC), (128, HC * TPC))
        in_maps.append({
            "xt": xt, "xc32": xc32, "masks": mk,
            "wih": wih, "whh": whh, "bias": biasv,
            "linw": linw, "linb": linb,
            "identb": identb, "ident32": ident32, "ones": ones,
            "onesr": np.ones((1, 512), bf),
        })
    return in_maps


def kernel(**inputs) -> np.ndarray:
    if "nc" not in _CACHE:
        _CACHE["nc"] = _build()
    nc = _CACHE["nc"]
    in_maps = _host_prep(inputs)
    res = run_bass_kernel_spmd(nc, in_maps, core_ids=list(range(NCORES)))
    out = np.empty((B, L, NL), np.float32)
    for core in range(NCORES):
        b = core // 2
        t0 = (core % 2) * TPC
        out[b, t0:t0 + TPC] = res.results[core]["out"].T
    return out

